# revision 56
# baseline (speedup 1.0000x reference)
"""Dense transformer block (QKV -> causal attention -> out-proj -> FFN+ReLU)
on 8 Trainium2 NeuronCores, data-parallel over the batch dimension.

Contract: kernel(**inputs) takes the FULL inputs
  x [8, 1024, 1024] f32, Wq/Wk/Wv/Wo/W1 [1024, 1024] f32, bo/b1 [1024] f32
and returns the FULL output [8, 1024, 1024] f32.

Each of the 8 cores runs the identical single-core program on one batch
element (batch=8, cores=8 -> no collectives needed).

Single-core design (bf16 tensor-engine compute, fp32 accumulation):
  - x arrives via casting SWDGE DMA (f32->bf16), emitted BEFORE the weight
    loads so it has descriptor-queue priority, then PE-transposed into
    feature-major xT [E, T].
  - qT, kT produced feature-major (lhsT=W, rhs=xT); score matmuls contract
    K=64 on each head's own 64-partition slice (offsets 0/64).
  - v is token-major "augmented": each head owns a 128-col block
    [ones col | 63 zero cols | 64 value cols], so the attn@v lhsT is a
    plain 2D slice and the attn output PSUM carries the softmax sum at
    ROW 0 (the only partition offset reciprocal_approx_fast supports) and
    the values at rows 64..127.
  - scores computed TRANSPOSED, s[t2, t1]; causal masking of the diagonal
    128x128 block is a PSUM-accumulated matmul of a -30000 upper-triangle
    constant (keeps the chain PE->scalar, no gpsimd hop); exp on scalar.
  - normalization: reciprocal_approx_fast reads the sum rows straight out
    of PSUM, one broadcast DMA per head, and a fused
    tensor_mul(PSUM, rb) -> bf16 SBUF eviction (no copy+cast+mul chain).
  - emission order is pipelined so the PE never drains: v chunks 4-7
    inside attention t1=0, out-proj/FFN for t1=0 inside attention t1=1;
    FFN stores are split across two DMA queues to shorten the tail.
    (PE clock p-states: the array only reaches full clock after ~3us of
    continuous execution, so gaps are doubly expensive.)
"""

import numpy as np
from contextlib import ExitStack

import concourse.bass as bass
import concourse.bacc as bacc
import concourse.tile as tile
from concourse import mybir
from concourse.bass_utils import run_bass_kernel_spmd

F32 = mybir.dt.float32
BF16 = mybir.dt.bfloat16

N_CORES = 8
BATCH = 8
T = 1024
E = 1024
H = 16
DH = 64


def build_nc(TT=T, EE=E, HH=H, Dh=DH):
    nc = bacc.Bacc("TRN2", target_bir_lowering=False, num_swdge_queues=4)

    x = nc.dram_tensor("x", [TT, EE], F32, kind="ExternalInput")
    Wq = nc.dram_tensor("Wq", [EE, EE], F32, kind="ExternalInput")
    Wk = nc.dram_tensor("Wk", [EE, EE], F32, kind="ExternalInput")
    Wv = nc.dram_tensor("Wv", [EE, EE], F32, kind="ExternalInput")
    Wo = nc.dram_tensor("Wo", [EE, EE], F32, kind="ExternalInput")
    bo = nc.dram_tensor("bo", [EE], F32, kind="ExternalInput")
    W1 = nc.dram_tensor("W1", [EE, EE], F32, kind="ExternalInput")
    b1 = nc.dram_tensor("b1", [EE], F32, kind="ExternalInput")
    out = nc.dram_tensor("out", [TT, EE], F32, kind="ExternalOutput")

    EC = EE // 128          # feature-chunk count (partition tiles)
    TC = TT // 128          # token-chunk count
    QT = min(512, TT)       # t1 (query) free-dim chunk
    NT = TT // QT
    QE = min(512, EE)       # output-feature free-dim chunk
    NE = EE // QE
    HP = 128 // Dh          # heads per 128-partition feature tile
    NP = HH // HP           # number of head pairs (= EC)
    scale = float(Dh) ** -0.5
    Exp = mybir.ActivationFunctionType.Exp
    Relu = mybir.ActivationFunctionType.Relu

    with ExitStack() as ctx:
        tc = ctx.enter_context(tile.TileContext(nc))
        wpool = ctx.enter_context(tc.tile_pool(name="w", bufs=3 * EC))
        xtokp = ctx.enter_context(tc.tile_pool(name="xtok", bufs=TC))
        xTp = ctx.enter_context(tc.tile_pool(name="xT", bufs=EC))
        qTp = ctx.enter_context(tc.tile_pool(name="qT", bufs=EC))
        kTp = ctx.enter_context(tc.tile_pool(name="kT", bufs=EC))
        vp = ctx.enter_context(tc.tile_pool(name="v", bufs=TC))
        pp = ctx.enter_context(tc.tile_pool(name="p", bufs=7))
        rtp = ctx.enter_context(tc.tile_pool(name="rt", bufs=2))
        rbp = ctx.enter_context(tc.tile_pool(name="rb", bufs=3))
        aoutp = ctx.enter_context(tc.tile_pool(name="aout", bufs=EC))
        projp = ctx.enter_context(tc.tile_pool(name="proj", bufs=EC))
        constp = ctx.enter_context(tc.tile_pool(name="const", bufs=1))
        ffoutp = ctx.enter_context(tc.tile_pool(name="ffout", bufs=3))
        ps_acc = ctx.enter_context(tc.tile_pool(name="ps_acc", bufs=2, space="PSUM"))
        ps_s = ctx.enter_context(tc.tile_pool(name="ps_s", bufs=2, space="PSUM"))
        ps_o = ctx.enter_context(tc.tile_pool(name="ps_o", bufs=4, space="PSUM"))

        # ---- constants ----
        bo_sb = constp.tile([128, EC], F32)
        nc.sync.dma_start(out=bo_sb, in_=bo.rearrange("(c p) -> p c", p=128))
        b1_sb = constp.tile([1, EE], BF16)
        nc.gpsimd.dma_start(out=b1_sb, in_=b1.rearrange("(a e) -> a e", a=1))
        ones_t = constp.tile([1, 128], BF16)
        nc.vector.memset(ones_t, 1.0)
        ident = constp.tile([128, 128], BF16)
        from concourse.masks import make_identity
        make_identity(nc, ident)
        # causal mask as a PSUM-accumulated matmul: matmul(lhsT=A, rhs=ident)
        # adds A^T to the score block. We want score[p, c] += -30000 where
        # c < p (future keys), so A[k, m] = -30000 where k < m: keep where
        # (c - p) <= 0, fill the rest.
        tri_negT = constp.tile([128, 128], BF16)
        nc.gpsimd.memset(tri_negT, 0.0)
        nc.gpsimd.affine_select(
            out=tri_negT, in_=tri_negT,
            compare_op=mybir.AluOpType.is_ge,
            fill=-30000.0, base=0, pattern=[[-1, 128]], channel_multiplier=1,
        )

        def load_w(wdram):
            tiles = []
            for ei in range(EC):
                wt = wpool.tile([128, EE], BF16, tag="w")
                nc.gpsimd.dma_start(out=wt, in_=wdram[128 * ei:128 * (ei + 1), :])
                tiles.append(wt)
            return tiles

        # ---- x: casting DMA (f32->bf16 SWDGE) then PE-transpose ----
        # x cast-DMAs are emitted BEFORE any weight load so the 4MB of x has
        # queue priority over 12MB of weight descriptors; the transposes (and
        # everything downstream) can then start as soon as chunk 0 lands.
        xT = [xTp.tile([128, TT], BF16, name="xT", tag="xT") for _ in range(EC)]
        xtoks = []
        for ti in range(TC):
            xtok = xtokp.tile([128, EE], BF16, tag="xtok")
            if ti == 0:
                # split the first chunk so transposes start after half lands
                half = EE // 2
                nc.gpsimd.dma_start(out=xtok[:, 0:half], in_=x[0:128, 0:half])
                nc.gpsimd.dma_start(out=xtok[:, half:EE], in_=x[0:128, half:EE])
            else:
                nc.gpsimd.dma_start(
                    out=xtok, in_=x[128 * ti:128 * (ti + 1), :]
                )
            xtoks.append(xtok)

        # wq/wk next (q-proj and k-proj are the first consumers)
        wq = load_w(Wq)
        wk = load_w(Wk)
        wv = load_w(Wv)

        def emit_xpose(tis):
            for ti in tis:
                for ec in range(EC):
                    ps_t = ps_acc.tile([128, 128], BF16, name="ps_t", tag="ps_acc")
                    nc.tensor.transpose(
                        ps_t, xtoks[ti][:, 128 * ec:128 * (ec + 1)], ident
                    )
                    dst = xT[ec][:, 128 * ti:128 * (ti + 1)]
                    if ec % 2 == 0:
                        nc.vector.tensor_copy(out=dst, in_=ps_t)
                    else:
                        nc.scalar.copy(out=dst, in_=ps_t)

        # ---- q/k: feature-major [128, T] per chunk, emitted per t1 half so
        # attention t1=0 (and its scalar exp stream) can start while the
        # t1=1 projections still run on the PE ----
        qT = [qTp.tile([128, TT], BF16, name="qT", tag="qT") for _ in range(EC)]
        kT = [kTp.tile([128, TT], BF16, name="kT", tag="kT") for _ in range(EC)]

        def emit_proj_half(wtiles, dst, t1, evict):
            for eo in range(EC):
                ps = ps_acc.tile([128, QT], F32, name="ps_acc", tag="ps_acc")
                for ei in range(EC):
                    nc.tensor.matmul(
                        ps,
                        lhsT=wtiles[ei][:, 128 * eo:128 * (eo + 1)],
                        rhs=xT[ei][:, QT * t1:QT * (t1 + 1)],
                        start=(ei == 0),
                        stop=(ei == EC - 1),
                    )
                evict(out=dst[eo][:, QT * t1:QT * (t1 + 1)], in_=ps)

        emit_xpose(range(TC))
        for t1h in range(NT):
            emit_proj_half(wq, qT, t1h, lambda out, in_: nc.vector.tensor_copy(out=out, in_=in_))
        for t1h in range(NT):
            emit_proj_half(wk, kT, t1h, lambda out, in_: nc.scalar.copy(out=out, in_=in_))

        # ---- v: token-major augmented, one 128-col block per head ----
        # head h occupies cols [128h, 128h+128): col 128h = ones (softmax
        # sum), cols +1..63 = zeros, cols +64..127 = the head's values. The
        # attn@v lhsT is the plain 2D slice va[:, 128h:128h+128], so the attn
        # output PSUM has the sum at ROW 0 (where reciprocal_approx_fast can
        # read it — the custom-DVE op only works at partition offset 0) and
        # the values at rows 64..127 (32-aligned for the fused mul).
        vaug = [None] * TC

        def emit_vchunk(ti, sink=None):
            """Emit the v-projection for token chunk ti. With sink=None all
            instructions are emitted inline; with a list, each matmul/evict
            is appended as a single-instruction thunk (PE filler)."""
            va = vp.tile([128, 2 * Dh * HH], BF16, name="va")
            ones_ap = bass.AP(
                tensor=va.tensor, offset=va.offset,
                ap=[list(va.ap[0]), [2 * Dh, HH], [1, 1]],
            )
            nc.gpsimd.memset(ones_ap, 1.0)
            zeros_ap = bass.AP(
                tensor=va.tensor, offset=va.offset + 1,
                ap=[list(va.ap[0]), [2 * Dh, HH], [1, Dh - 1]],
            )
            nc.gpsimd.memset(zeros_ap, 0.0)
            vaug[ti] = va
            for eoq in range(NE):
                box = {}

                def mm(ei, eoq=eoq, box=box):
                    if ei == 0:
                        box["ps"] = ps_acc.tile(
                            [128, QE], F32, name="ps_acc", tag="ps_acc"
                        )
                    nc.tensor.matmul(
                        box["ps"],
                        lhsT=xT[ei][:, 128 * ti:128 * (ti + 1)],
                        rhs=wv[ei][:, QE * eoq:QE * (eoq + 1)],
                        start=(ei == 0),
                        stop=(ei == EC - 1),
                    )

                def ev(eoq=eoq, box=box):
                    hq = QE // Dh
                    dst = va[:, 2 * Dh * hq * eoq:2 * Dh * hq * (eoq + 1)]
                    dst = dst.rearrange("p (h c) -> p h c", c=2 * Dh)[:, :, Dh:2 * Dh]
                    src = box["ps"].rearrange("p (h d) -> p h d", d=Dh)
                    nc.scalar.copy(out=dst, in_=src)

                thunks = [lambda ei=ei, mm=mm: mm(ei) for ei in range(EC)] + [ev]
                if sink is None:
                    for t in thunks:
                        t()
                else:
                    sink.extend(thunks)

        # v chunks needed by attention t1=0 (t2 blocks 0..QT/128-1)
        for ti in range(QT // 128):
            emit_vchunk(ti, sink=None)

        # prefetch out-proj / FFN weights (consumed mid-attention)
        wo = load_w(Wo)
        w1 = load_w(W1)

        # ---- attention + interleaved out-proj/FFN ----
        aoutT = [aoutp.tile([128, TT], BF16, name="aoutT", tag="aoutT") for _ in range(EC)]
        projT = [projp.tile([128, TT], BF16, name="projT", tag="projT") for _ in range(EC)]

        def emit_outproj(eo, t1, sink=None, pool=None, tag="ps_acc"):
            box = {}

            def mm(ei):
                if ei == 0:
                    box["ps"] = (pool or ps_acc).tile(
                        [128, QT], F32, name="ps_acc", tag=tag
                    )
                nc.tensor.matmul(
                    box["ps"],
                    lhsT=wo[ei][:, 128 * eo:128 * (eo + 1)],
                    rhs=aoutT[ei][:, QT * t1:QT * (t1 + 1)],
                    start=(ei == 0),
                    stop=(ei == EC - 1),
                )

            def ev():
                nc.vector.tensor_scalar_add(
                    out=projT[eo][:, QT * t1:QT * (t1 + 1)],
                    in0=box["ps"],
                    scalar1=bo_sb[:, eo:eo + 1],
                )

            thunks = [lambda ei=ei: mm(ei) for ei in range(EC)] + [ev]
            if sink is None:
                for t in thunks:
                    t()
            else:
                sink.extend(thunks)

        def emit_ffn(ti, sink=None, pool=None, tag="ps_acc"):
            for eoq in range(NE):
                box = {}

                def mm(ei, eoq=eoq, box=box):
                    if ei == 0:
                        box["ps"] = (pool or ps_acc).tile(
                            [128, QE], F32, name="ps_acc", tag=tag
                        )
                    nc.tensor.matmul(
                        box["ps"],
                        lhsT=projT[ei][:, 128 * ti:128 * (ti + 1)],
                        rhs=w1[ei][:, QE * eoq:QE * (eoq + 1)],
                        start=(ei == 0),
                        stop=False,
                    )

                def bias(eoq=eoq, box=box):
                    nc.tensor.matmul(
                        box["ps"],
                        lhsT=ones_t[:, 0:128],
                        rhs=b1_sb[:, QE * eoq:QE * (eoq + 1)],
                        start=False,
                        stop=True,
                    )

                def ev(eoq=eoq, box=box):
                    fo = ffoutp.tile([128, QE], F32)
                    nc.scalar.activation(out=fo, in_=box["ps"], func=Relu)
                    # split the 256KB store across two queues (one queue
                    # moves ~22.5 GB/s, so an unsplit store costs ~11.6us
                    # and would serialize the kernel tail)
                    h0 = QE // 2
                    engs = [nc.sync, nc.gpsimd]
                    for s, eng in enumerate(
                        [engs[(2 * ti + eoq) % 2], engs[(2 * ti + eoq + 1) % 2]]
                    ):
                        eng.dma_start(
                            out=out[128 * ti:128 * (ti + 1),
                                    QE * eoq + s * h0:QE * eoq + (s + 1) * h0],
                            in_=fo[:, s * h0:(s + 1) * h0],
                        )

                thunks = [lambda ei=ei, mm=mm: mm(ei) for ei in range(EC)] + [bias, ev]
                if sink is None:
                    for t in thunks:
                        t()
                else:
                    sink.extend(thunks)

        for t1 in range(NT):
            t2cs = [t2 for t2 in range(TC) if 128 * t2 < QT * (t1 + 1)]
            for p in range(NP):
                pair = [HP * p + i for i in range(HP)]
                opss = [ps_o.tile([128, QT], F32, name="ops", tag="ops")
                        for _ in pair]
                for j, t2 in enumerate(t2cs):
                    k0 = 128 * t2 - QT * t1
                    c0 = max(0, k0)
                    for hi, h in enumerate(pair):
                        po = hi * Dh
                        sp = ps_s.tile([128, QT], F32, name="sp", tag="sp")
                        diag = k0 >= 0
                        nc.tensor.matmul(
                            sp[:, c0:QT],
                            lhsT=kT[p][po:po + Dh, 128 * t2:128 * (t2 + 1)],
                            rhs=qT[p][po:po + Dh, QT * t1 + c0:QT * (t1 + 1)],
                            start=True,
                            stop=not diag,
                        )
                        if diag:
                            # diagonal 128x128 sub-block: add -30000 to
                            # future pairs in PSUM (exp then yields 0),
                            # keeping the chain PE->scalar with no gpsimd hop
                            nc.tensor.matmul(
                                sp[:, c0:c0 + 128],
                                lhsT=tri_negT,
                                rhs=ident,
                                start=False,
                                stop=True,
                            )
                        pt = pp.tile([128, QT], BF16)
                        nc.scalar.activation(
                            out=pt[:, c0:QT], in_=sp[:, c0:QT],
                            func=Exp, scale=scale,
                        )
                        va_h = vaug[t2][:, 128 * h:128 * (h + 1)]
                        nc.tensor.matmul(
                            opss[hi][:, c0:QT],
                            lhsT=va_h,
                            rhs=pt[:, c0:QT],
                            start=(j == 0),
                            stop=(j == len(t2cs) - 1),
                        )
                # normalization: recip of the PSUM sum rows (row 0, where the
                # custom-DVE op works), per-head broadcast DMA, fused
                # evict-multiply straight from PSUM to bf16 SBUF.
                rts = rtp.tile([1, 2 * QT], F32, name="rts", tag="rts")
                rb = rbp.tile([128, QT], F32)
                for hi in range(HP):
                    nc.vector.reciprocal_approx_fast(
                        out=rts[0:1, QT * hi:QT * (hi + 1)],
                        in_=opss[hi][0:1, :],
                    )
                    r_h = rts[0:1, QT * hi:QT * (hi + 1)]
                    r_src = bass.AP(
                        tensor=r_h.tensor,
                        offset=r_h.offset,
                        ap=[list(r_h.ap[0]), [0, Dh]] + list(r_h.ap[1:]),
                    )
                    nc.sync.dma_start(
                        out=rb[Dh * hi:Dh * (hi + 1), :], in_=r_src
                    )
                for hi in range(HP):
                    po = hi * Dh
                    nc.vector.tensor_mul(
                        out=aoutT[p][po:po + Dh, QT * t1:QT * (t1 + 1)],
                        in0=opss[hi][Dh:2 * Dh, :],
                        in1=rb[po:po + Dh, :],
                    )
                # interleave other work into the attention stream
                if t1 == 0:
                    vnext = QT // 128 + p
                    if vnext < TC:
                        emit_vchunk(vnext)
                elif t1 == NT - 1:
                    if 2 * p + 1 < EC:
                        emit_outproj(2 * p, 0)
                        emit_outproj(2 * p + 1, 0)
                    else:
                        emit_ffn(p - NP // 2)

        # ---- tail: out-proj + FFN for the last t1 block ----
        # Attention PSUM banks are idle now: out-proj t1=1 alternates
        # ps_acc/ps_s (4 banks, no eviction-WAR stalls) while FFN chunks
        # 4 and 5 accumulate incrementally on the 4 ps_o banks, consuming
        # each projT[ei] as soon as its out-proj eviction lands.
        t1l = NT - 1
        tis = list(range(QT // 128, TC))
        early = tis[:2]  # chunks accumulated incrementally on ps_o
        fboxes = {}

        def ffn_mm(ti, eoq, ei):
            if ei == 0:
                fboxes[(ti, eoq)] = ps_o.tile(
                    [128, QE], F32, name="ps_f", tag="ops"
                )
            nc.tensor.matmul(
                fboxes[(ti, eoq)],
                lhsT=projT[ei][:, 128 * ti:128 * (ti + 1)],
                rhs=w1[ei][:, QE * eoq:QE * (eoq + 1)],
                start=(ei == 0),
                stop=False,
            )

        # head-start: both of the first two out-proj groups accumulate
        # ei=0..6 BEFORE either touches ei=7 (which depends on the last
        # attention pair's normalization chain) — ~3us of ready PE work
        # runs while that chain completes.
        obox = {}

        def op_mm(k, ei):
            if ei == 0:
                obox[k] = (ps_acc if k % 2 == 0 else ps_s).tile(
                    [128, QT], F32, name="ps_acc",
                    tag=("ps_acc" if k % 2 == 0 else "sp"),
                )
            nc.tensor.matmul(
                obox[k],
                lhsT=wo[ei][:, 128 * k:128 * (k + 1)],
                rhs=aoutT[ei][:, QT * t1l:QT * (t1l + 1)],
                start=(ei == 0),
                stop=(ei == EC - 1),
            )

        for k in (0, 1):
            for ei in range(EC - 1):
                op_mm(k, ei)
        for k in (0, 1):
            op_mm(k, EC - 1)
            nc.vector.tensor_scalar_add(
                out=projT[k][:, QT * t1l:QT * (t1l + 1)],
                in0=obox[k],
                scalar1=bo_sb[:, k:k + 1],
            )
        for ti in early:
            for eoq in range(NE):
                ffn_mm(ti, eoq, 0)
        for k in range(2, EC):
            emit_outproj(
                k, t1l,
                pool=(ps_acc if k % 2 == 0 else ps_s),
                tag=("ps_acc" if k % 2 == 0 else "sp"),
            )
            for ti in early:
                for eoq in range(NE):
                    ffn_mm(ti, eoq, k - 1)
        for ti in early:
            for eoq in range(NE):
                ffn_mm(ti, eoq, EC - 1)
                nc.tensor.matmul(
                    fboxes[(ti, eoq)],
                    lhsT=ones_t[:, 0:128],
                    rhs=b1_sb[:, QE * eoq:QE * (eoq + 1)],
                    start=False,
                    stop=True,
                )
                fo = ffoutp.tile([128, QE], F32)
                nc.scalar.activation(out=fo, in_=fboxes[(ti, eoq)], func=Relu)
                h0 = QE // 2
                engs = [nc.sync, nc.gpsimd]
                for s, eng in enumerate(
                    [engs[(2 * ti + eoq) % 2], engs[(2 * ti + eoq + 1) % 2]]
                ):
                    eng.dma_start(
                        out=out[128 * ti:128 * (ti + 1),
                                QE * eoq + s * h0:QE * eoq + (s + 1) * h0],
                        in_=fo[:, s * h0:(s + 1) * h0],
                    )
        for i, ti in enumerate(tis[2:]):
            emit_ffn(
                ti,
                pool=(ps_acc if i % 2 == 0 else ps_s),
                tag=("ps_acc" if i % 2 == 0 else "sp"),
            )

    nc.finalize()
    return nc


_NC_CACHE = {}


def _get_nc(shape_key):
    if shape_key not in _NC_CACHE:
        _NC_CACHE[shape_key] = build_nc(*shape_key)
    return _NC_CACHE[shape_key]


def kernel(x, Wq, Wk, Wv, Wo, bo, W1, b1):
    x = np.ascontiguousarray(np.asarray(x, dtype=np.float32))
    ws = {
        "Wq": np.ascontiguousarray(np.asarray(Wq, dtype=np.float32)),
        "Wk": np.ascontiguousarray(np.asarray(Wk, dtype=np.float32)),
        "Wv": np.ascontiguousarray(np.asarray(Wv, dtype=np.float32)),
        "Wo": np.ascontiguousarray(np.asarray(Wo, dtype=np.float32)),
        "bo": np.ascontiguousarray(np.asarray(bo, dtype=np.float32)),
        "W1": np.ascontiguousarray(np.asarray(W1, dtype=np.float32)),
        "b1": np.ascontiguousarray(np.asarray(b1, dtype=np.float32)),
    }
    B, TT, EE = x.shape
    assert B == N_CORES
    nc = _get_nc((TT, EE, H, DH))
    in_maps = [dict(ws, x=x[b]) for b in range(B)]
    res = run_bass_kernel_spmd(nc, in_maps, core_ids=list(range(N_CORES)))
    return np.stack([res.results[b]["out"] for b in range(B)], axis=0).astype(
        np.float32
    )


# revision 57
# speedup vs baseline: 1.0144x; 1.0144x over previous
"""Dense transformer block (QKV -> causal attention -> out-proj -> FFN+ReLU)
on 8 Trainium2 NeuronCores, data-parallel over the batch dimension.

Contract: kernel(**inputs) takes the FULL inputs
  x [8, 1024, 1024] f32, Wq/Wk/Wv/Wo/W1 [1024, 1024] f32, bo/b1 [1024] f32
and returns the FULL output [8, 1024, 1024] f32.

Each of the 8 cores runs the identical single-core program on one batch
element (batch=8, cores=8 -> no collectives needed).

Single-core design (bf16 tensor-engine compute, fp32 accumulation):
  - x arrives via casting SWDGE DMA (f32->bf16), emitted BEFORE the weight
    loads so it has descriptor-queue priority, then PE-transposed into
    feature-major xT [E, T].
  - qT, kT produced feature-major (lhsT=W, rhs=xT); score matmuls contract
    K=64 on each head's own 64-partition slice (offsets 0/64).
  - v is token-major "augmented": each head owns a 128-col block
    [ones col | 63 zero cols | 64 value cols], so the attn@v lhsT is a
    plain 2D slice and the attn output PSUM carries the softmax sum at
    ROW 0 (the only partition offset reciprocal_approx_fast supports) and
    the values at rows 64..127.
  - scores computed TRANSPOSED, s[t2, t1]; causal masking of the diagonal
    128x128 block is a PSUM-accumulated matmul of a -30000 upper-triangle
    constant (keeps the chain PE->scalar, no gpsimd hop); exp on scalar.
  - normalization: reciprocal_approx_fast reads the sum rows straight out
    of PSUM, one broadcast DMA per head, and a fused
    tensor_mul(PSUM, rb) -> bf16 SBUF eviction (no copy+cast+mul chain).
  - emission order is pipelined so the PE never drains: v chunks 4-7
    inside attention t1=0, out-proj/FFN for t1=0 inside attention t1=1;
    FFN stores are split across two DMA queues to shorten the tail.
    (PE clock p-states: the array only reaches full clock after ~3us of
    continuous execution, so gaps are doubly expensive.)
"""

import numpy as np
from contextlib import ExitStack

import concourse.bass as bass
import concourse.bacc as bacc
import concourse.tile as tile
from concourse import mybir
from concourse.bass_utils import run_bass_kernel_spmd

F32 = mybir.dt.float32
BF16 = mybir.dt.bfloat16

N_CORES = 8
BATCH = 8
T = 1024
E = 1024
H = 16
DH = 64


def build_nc(TT=T, EE=E, HH=H, Dh=DH):
    nc = bacc.Bacc("TRN2", target_bir_lowering=False, num_swdge_queues=4)

    x = nc.dram_tensor("x", [TT, EE], F32, kind="ExternalInput")
    Wq = nc.dram_tensor("Wq", [EE, EE], F32, kind="ExternalInput")
    Wk = nc.dram_tensor("Wk", [EE, EE], F32, kind="ExternalInput")
    Wv = nc.dram_tensor("Wv", [EE, EE], F32, kind="ExternalInput")
    Wo = nc.dram_tensor("Wo", [EE, EE], F32, kind="ExternalInput")
    bo = nc.dram_tensor("bo", [EE], F32, kind="ExternalInput")
    W1 = nc.dram_tensor("W1", [EE, EE], F32, kind="ExternalInput")
    b1 = nc.dram_tensor("b1", [EE], F32, kind="ExternalInput")
    out = nc.dram_tensor("out", [TT, EE], F32, kind="ExternalOutput")

    EC = EE // 128          # feature-chunk count (partition tiles)
    TC = TT // 128          # token-chunk count
    QT = min(512, TT)       # t1 (query) free-dim chunk
    NT = TT // QT
    QE = min(512, EE)       # output-feature free-dim chunk
    NE = EE // QE
    HP = 128 // Dh          # heads per 128-partition feature tile
    NP = HH // HP           # number of head pairs (= EC)
    scale = float(Dh) ** -0.5
    Exp = mybir.ActivationFunctionType.Exp
    Relu = mybir.ActivationFunctionType.Relu

    with ExitStack() as ctx:
        tc = ctx.enter_context(tile.TileContext(nc))
        wpool = ctx.enter_context(tc.tile_pool(name="w", bufs=3 * EC))
        xtokp = ctx.enter_context(tc.tile_pool(name="xtok", bufs=TC))
        xTp = ctx.enter_context(tc.tile_pool(name="xT", bufs=EC))
        qTp = ctx.enter_context(tc.tile_pool(name="qT", bufs=EC))
        kTp = ctx.enter_context(tc.tile_pool(name="kT", bufs=EC))
        vp = ctx.enter_context(tc.tile_pool(name="v", bufs=TC))
        pp = ctx.enter_context(tc.tile_pool(name="p", bufs=7))
        rtp = ctx.enter_context(tc.tile_pool(name="rt", bufs=2))
        rbp = ctx.enter_context(tc.tile_pool(name="rb", bufs=3))
        aoutp = ctx.enter_context(tc.tile_pool(name="aout", bufs=EC))
        projp = ctx.enter_context(tc.tile_pool(name="proj", bufs=EC))
        constp = ctx.enter_context(tc.tile_pool(name="const", bufs=1))
        ffoutp = ctx.enter_context(tc.tile_pool(name="ffout", bufs=3))
        ps_acc = ctx.enter_context(tc.tile_pool(name="ps_acc", bufs=2, space="PSUM"))
        ps_s = ctx.enter_context(tc.tile_pool(name="ps_s", bufs=2, space="PSUM"))
        ps_o = ctx.enter_context(tc.tile_pool(name="ps_o", bufs=4, space="PSUM"))

        # ---- constants ----
        bo_sb = constp.tile([128, EC], F32)
        nc.sync.dma_start(out=bo_sb, in_=bo.rearrange("(c p) -> p c", p=128))
        b1_sb = constp.tile([1, EE], BF16)
        nc.gpsimd.dma_start(out=b1_sb, in_=b1.rearrange("(a e) -> a e", a=1))
        ones_t = constp.tile([1, 128], BF16)
        nc.vector.memset(ones_t, 1.0)
        ident = constp.tile([128, 128], BF16)
        from concourse.masks import make_identity
        make_identity(nc, ident)
        # causal mask as a PSUM-accumulated matmul: matmul(lhsT=A, rhs=ident)
        # adds A^T to the score block. We want score[p, c] += -30000 where
        # c < p (future keys), so A[k, m] = -30000 where k < m: keep where
        # (c - p) <= 0, fill the rest.
        tri_negT = constp.tile([128, 128], BF16)
        nc.gpsimd.memset(tri_negT, 0.0)
        nc.gpsimd.affine_select(
            out=tri_negT, in_=tri_negT,
            compare_op=mybir.AluOpType.is_ge,
            fill=-30000.0, base=0, pattern=[[-1, 128]], channel_multiplier=1,
        )

        def load_w(wdram):
            tiles = []
            for ei in range(EC):
                wt = wpool.tile([128, EE], BF16, tag="w")
                nc.gpsimd.dma_start(out=wt, in_=wdram[128 * ei:128 * (ei + 1), :])
                tiles.append(wt)
            return tiles

        # PE warm-up: the tensor engine's clock ramps to full speed only
        # after ~3us of continuous execution. Burn the x-DMA lead-in (when
        # the PE would idle at the slowest p-state anyway) on dummy
        # transposes of the identity constant so the real transposes and
        # projections enter an already-ramped array.
        for _ in range(28):
            wps = ps_s.tile([128, 128], BF16, name="warm", tag="sp")
            nc.tensor.transpose(wps, ident, ident)

        # ---- x: casting DMA (f32->bf16 SWDGE) then PE-transpose ----
        # x cast-DMAs are emitted BEFORE any weight load so the 4MB of x has
        # queue priority over 12MB of weight descriptors; the transposes (and
        # everything downstream) can then start as soon as chunk 0 lands.
        xT = [xTp.tile([128, TT], BF16, name="xT", tag="xT") for _ in range(EC)]
        xtoks = []
        for ti in range(TC):
            xtok = xtokp.tile([128, EE], BF16, tag="xtok")
            if ti == 0:
                # split the first chunk so transposes start after half lands
                half = EE // 2
                nc.gpsimd.dma_start(out=xtok[:, 0:half], in_=x[0:128, 0:half])
                nc.gpsimd.dma_start(out=xtok[:, half:EE], in_=x[0:128, half:EE])
            else:
                nc.gpsimd.dma_start(
                    out=xtok, in_=x[128 * ti:128 * (ti + 1), :]
                )
            xtoks.append(xtok)

        # wq/wk next (q-proj and k-proj are the first consumers)
        wq = load_w(Wq)
        wk = load_w(Wk)
        wv = load_w(Wv)

        def emit_xpose(tis):
            for ti in tis:
                for ec in range(EC):
                    ps_t = ps_acc.tile([128, 128], BF16, name="ps_t", tag="ps_acc")
                    nc.tensor.transpose(
                        ps_t, xtoks[ti][:, 128 * ec:128 * (ec + 1)], ident
                    )
                    dst = xT[ec][:, 128 * ti:128 * (ti + 1)]
                    if ec % 2 == 0:
                        nc.vector.tensor_copy(out=dst, in_=ps_t)
                    else:
                        nc.scalar.copy(out=dst, in_=ps_t)

        # ---- q/k: feature-major [128, T] per chunk, emitted per t1 half so
        # attention t1=0 (and its scalar exp stream) can start while the
        # t1=1 projections still run on the PE ----
        qT = [qTp.tile([128, TT], BF16, name="qT", tag="qT") for _ in range(EC)]
        kT = [kTp.tile([128, TT], BF16, name="kT", tag="kT") for _ in range(EC)]

        def emit_proj_half(wtiles, dst, t1, evict):
            for eo in range(EC):
                ps = ps_acc.tile([128, QT], F32, name="ps_acc", tag="ps_acc")
                for ei in range(EC):
                    nc.tensor.matmul(
                        ps,
                        lhsT=wtiles[ei][:, 128 * eo:128 * (eo + 1)],
                        rhs=xT[ei][:, QT * t1:QT * (t1 + 1)],
                        start=(ei == 0),
                        stop=(ei == EC - 1),
                    )
                evict(out=dst[eo][:, QT * t1:QT * (t1 + 1)], in_=ps)

        emit_xpose(range(TC))
        for t1h in range(NT):
            emit_proj_half(wq, qT, t1h, lambda out, in_: nc.vector.tensor_copy(out=out, in_=in_))
        for t1h in range(NT):
            emit_proj_half(wk, kT, t1h, lambda out, in_: nc.scalar.copy(out=out, in_=in_))

        # ---- v: token-major augmented, one 128-col block per head ----
        # head h occupies cols [128h, 128h+128): col 128h = ones (softmax
        # sum), cols +1..63 = zeros, cols +64..127 = the head's values. The
        # attn@v lhsT is the plain 2D slice va[:, 128h:128h+128], so the attn
        # output PSUM has the sum at ROW 0 (where reciprocal_approx_fast can
        # read it — the custom-DVE op only works at partition offset 0) and
        # the values at rows 64..127 (32-aligned for the fused mul).
        vaug = [None] * TC

        def emit_vchunk(ti, sink=None):
            """Emit the v-projection for token chunk ti. With sink=None all
            instructions are emitted inline; with a list, each matmul/evict
            is appended as a single-instruction thunk (PE filler)."""
            va = vp.tile([128, 2 * Dh * HH], BF16, name="va")
            ones_ap = bass.AP(
                tensor=va.tensor, offset=va.offset,
                ap=[list(va.ap[0]), [2 * Dh, HH], [1, 1]],
            )
            nc.gpsimd.memset(ones_ap, 1.0)
            zeros_ap = bass.AP(
                tensor=va.tensor, offset=va.offset + 1,
                ap=[list(va.ap[0]), [2 * Dh, HH], [1, Dh - 1]],
            )
            nc.gpsimd.memset(zeros_ap, 0.0)
            vaug[ti] = va
            for eoq in range(NE):
                box = {}

                def mm(ei, eoq=eoq, box=box):
                    if ei == 0:
                        box["ps"] = ps_acc.tile(
                            [128, QE], F32, name="ps_acc", tag="ps_acc"
                        )
                    nc.tensor.matmul(
                        box["ps"],
                        lhsT=xT[ei][:, 128 * ti:128 * (ti + 1)],
                        rhs=wv[ei][:, QE * eoq:QE * (eoq + 1)],
                        start=(ei == 0),
                        stop=(ei == EC - 1),
                    )

                def ev(eoq=eoq, box=box):
                    hq = QE // Dh
                    dst = va[:, 2 * Dh * hq * eoq:2 * Dh * hq * (eoq + 1)]
                    dst = dst.rearrange("p (h c) -> p h c", c=2 * Dh)[:, :, Dh:2 * Dh]
                    src = box["ps"].rearrange("p (h d) -> p h d", d=Dh)
                    nc.scalar.copy(out=dst, in_=src)

                thunks = [lambda ei=ei, mm=mm: mm(ei) for ei in range(EC)] + [ev]
                if sink is None:
                    for t in thunks:
                        t()
                else:
                    sink.extend(thunks)

        # v chunks needed by attention t1=0 (t2 blocks 0..QT/128-1)
        for ti in range(QT // 128):
            emit_vchunk(ti, sink=None)

        # prefetch out-proj / FFN weights (consumed mid-attention)
        wo = load_w(Wo)
        w1 = load_w(W1)

        # ---- attention + interleaved out-proj/FFN ----
        aoutT = [aoutp.tile([128, TT], BF16, name="aoutT", tag="aoutT") for _ in range(EC)]
        projT = [projp.tile([128, TT], BF16, name="projT", tag="projT") for _ in range(EC)]

        def emit_outproj(eo, t1, sink=None, pool=None, tag="ps_acc"):
            box = {}

            def mm(ei):
                if ei == 0:
                    box["ps"] = (pool or ps_acc).tile(
                        [128, QT], F32, name="ps_acc", tag=tag
                    )
                nc.tensor.matmul(
                    box["ps"],
                    lhsT=wo[ei][:, 128 * eo:128 * (eo + 1)],
                    rhs=aoutT[ei][:, QT * t1:QT * (t1 + 1)],
                    start=(ei == 0),
                    stop=(ei == EC - 1),
                )

            def ev():
                nc.vector.tensor_scalar_add(
                    out=projT[eo][:, QT * t1:QT * (t1 + 1)],
                    in0=box["ps"],
                    scalar1=bo_sb[:, eo:eo + 1],
                )

            thunks = [lambda ei=ei: mm(ei) for ei in range(EC)] + [ev]
            if sink is None:
                for t in thunks:
                    t()
            else:
                sink.extend(thunks)

        def emit_ffn(ti, sink=None, pool=None, tag="ps_acc"):
            for eoq in range(NE):
                box = {}

                def mm(ei, eoq=eoq, box=box):
                    if ei == 0:
                        box["ps"] = (pool or ps_acc).tile(
                            [128, QE], F32, name="ps_acc", tag=tag
                        )
                    nc.tensor.matmul(
                        box["ps"],
                        lhsT=projT[ei][:, 128 * ti:128 * (ti + 1)],
                        rhs=w1[ei][:, QE * eoq:QE * (eoq + 1)],
                        start=(ei == 0),
                        stop=False,
                    )

                def bias(eoq=eoq, box=box):
                    nc.tensor.matmul(
                        box["ps"],
                        lhsT=ones_t[:, 0:128],
                        rhs=b1_sb[:, QE * eoq:QE * (eoq + 1)],
                        start=False,
                        stop=True,
                    )

                def ev(eoq=eoq, box=box):
                    fo = ffoutp.tile([128, QE], F32)
                    nc.scalar.activation(out=fo, in_=box["ps"], func=Relu)
                    # split the 256KB store across two queues (one queue
                    # moves ~22.5 GB/s, so an unsplit store costs ~11.6us
                    # and would serialize the kernel tail)
                    h0 = QE // 2
                    engs = [nc.sync, nc.gpsimd]
                    for s, eng in enumerate(
                        [engs[(2 * ti + eoq) % 2], engs[(2 * ti + eoq + 1) % 2]]
                    ):
                        eng.dma_start(
                            out=out[128 * ti:128 * (ti + 1),
                                    QE * eoq + s * h0:QE * eoq + (s + 1) * h0],
                            in_=fo[:, s * h0:(s + 1) * h0],
                        )

                thunks = [lambda ei=ei, mm=mm: mm(ei) for ei in range(EC)] + [bias, ev]
                if sink is None:
                    for t in thunks:
                        t()
                else:
                    sink.extend(thunks)

        for t1 in range(NT):
            t2cs = [t2 for t2 in range(TC) if 128 * t2 < QT * (t1 + 1)]
            for p in range(NP):
                pair = [HP * p + i for i in range(HP)]
                opss = [ps_o.tile([128, QT], F32, name="ops", tag="ops")
                        for _ in pair]
                for j, t2 in enumerate(t2cs):
                    k0 = 128 * t2 - QT * t1
                    c0 = max(0, k0)
                    for hi, h in enumerate(pair):
                        po = hi * Dh
                        sp = ps_s.tile([128, QT], F32, name="sp", tag="sp")
                        diag = k0 >= 0
                        nc.tensor.matmul(
                            sp[:, c0:QT],
                            lhsT=kT[p][po:po + Dh, 128 * t2:128 * (t2 + 1)],
                            rhs=qT[p][po:po + Dh, QT * t1 + c0:QT * (t1 + 1)],
                            start=True,
                            stop=not diag,
                        )
                        if diag:
                            # diagonal 128x128 sub-block: add -30000 to
                            # future pairs in PSUM (exp then yields 0),
                            # keeping the chain PE->scalar with no gpsimd hop
                            nc.tensor.matmul(
                                sp[:, c0:c0 + 128],
                                lhsT=tri_negT,
                                rhs=ident,
                                start=False,
                                stop=True,
                            )
                        pt = pp.tile([128, QT], BF16)
                        nc.scalar.activation(
                            out=pt[:, c0:QT], in_=sp[:, c0:QT],
                            func=Exp, scale=scale,
                        )
                        va_h = vaug[t2][:, 128 * h:128 * (h + 1)]
                        nc.tensor.matmul(
                            opss[hi][:, c0:QT],
                            lhsT=va_h,
                            rhs=pt[:, c0:QT],
                            start=(j == 0),
                            stop=(j == len(t2cs) - 1),
                        )
                # normalization: recip of the PSUM sum rows (row 0, where the
                # custom-DVE op works), per-head broadcast DMA, fused
                # evict-multiply straight from PSUM to bf16 SBUF.
                rts = rtp.tile([1, 2 * QT], F32, name="rts", tag="rts")
                rb = rbp.tile([128, QT], F32)
                for hi in range(HP):
                    nc.vector.reciprocal_approx_fast(
                        out=rts[0:1, QT * hi:QT * (hi + 1)],
                        in_=opss[hi][0:1, :],
                    )
                    r_h = rts[0:1, QT * hi:QT * (hi + 1)]
                    r_src = bass.AP(
                        tensor=r_h.tensor,
                        offset=r_h.offset,
                        ap=[list(r_h.ap[0]), [0, Dh]] + list(r_h.ap[1:]),
                    )
                    nc.sync.dma_start(
                        out=rb[Dh * hi:Dh * (hi + 1), :], in_=r_src
                    )
                for hi in range(HP):
                    po = hi * Dh
                    nc.vector.tensor_mul(
                        out=aoutT[p][po:po + Dh, QT * t1:QT * (t1 + 1)],
                        in0=opss[hi][Dh:2 * Dh, :],
                        in1=rb[po:po + Dh, :],
                    )
                # interleave other work into the attention stream
                if t1 == 0:
                    vnext = QT // 128 + p
                    if vnext < TC:
                        emit_vchunk(vnext)
                elif t1 == NT - 1:
                    if 2 * p + 1 < EC:
                        emit_outproj(2 * p, 0)
                        emit_outproj(2 * p + 1, 0)
                    else:
                        emit_ffn(p - NP // 2)

        # ---- tail: out-proj + FFN for the last t1 block ----
        # Attention PSUM banks are idle now: out-proj t1=1 alternates
        # ps_acc/ps_s (4 banks, no eviction-WAR stalls) while FFN chunks
        # 4 and 5 accumulate incrementally on the 4 ps_o banks, consuming
        # each projT[ei] as soon as its out-proj eviction lands.
        t1l = NT - 1
        tis = list(range(QT // 128, TC))
        early = tis[:2]  # chunks accumulated incrementally on ps_o
        fboxes = {}

        def ffn_mm(ti, eoq, ei):
            if ei == 0:
                fboxes[(ti, eoq)] = ps_o.tile(
                    [128, QE], F32, name="ps_f", tag="ops"
                )
            nc.tensor.matmul(
                fboxes[(ti, eoq)],
                lhsT=projT[ei][:, 128 * ti:128 * (ti + 1)],
                rhs=w1[ei][:, QE * eoq:QE * (eoq + 1)],
                start=(ei == 0),
                stop=False,
            )

        # head-start: both of the first two out-proj groups accumulate
        # ei=0..6 BEFORE either touches ei=7 (which depends on the last
        # attention pair's normalization chain) — ~3us of ready PE work
        # runs while that chain completes.
        obox = {}

        def op_mm(k, ei):
            if ei == 0:
                obox[k] = (ps_acc if k % 2 == 0 else ps_s).tile(
                    [128, QT], F32, name="ps_acc",
                    tag=("ps_acc" if k % 2 == 0 else "sp"),
                )
            nc.tensor.matmul(
                obox[k],
                lhsT=wo[ei][:, 128 * k:128 * (k + 1)],
                rhs=aoutT[ei][:, QT * t1l:QT * (t1l + 1)],
                start=(ei == 0),
                stop=(ei == EC - 1),
            )

        for k in (0, 1):
            for ei in range(EC - 1):
                op_mm(k, ei)
        for k in (0, 1):
            op_mm(k, EC - 1)
            nc.vector.tensor_scalar_add(
                out=projT[k][:, QT * t1l:QT * (t1l + 1)],
                in0=obox[k],
                scalar1=bo_sb[:, k:k + 1],
            )
        for ti in early:
            for eoq in range(NE):
                ffn_mm(ti, eoq, 0)
        for k in range(2, EC):
            emit_outproj(
                k, t1l,
                pool=(ps_acc if k % 2 == 0 else ps_s),
                tag=("ps_acc" if k % 2 == 0 else "sp"),
            )
            for ti in early:
                for eoq in range(NE):
                    ffn_mm(ti, eoq, k - 1)
        for ti in early:
            for eoq in range(NE):
                ffn_mm(ti, eoq, EC - 1)
                nc.tensor.matmul(
                    fboxes[(ti, eoq)],
                    lhsT=ones_t[:, 0:128],
                    rhs=b1_sb[:, QE * eoq:QE * (eoq + 1)],
                    start=False,
                    stop=True,
                )
                fo = ffoutp.tile([128, QE], F32)
                nc.scalar.activation(out=fo, in_=fboxes[(ti, eoq)], func=Relu)
                h0 = QE // 2
                engs = [nc.sync, nc.gpsimd]
                for s, eng in enumerate(
                    [engs[(2 * ti + eoq) % 2], engs[(2 * ti + eoq + 1) % 2]]
                ):
                    eng.dma_start(
                        out=out[128 * ti:128 * (ti + 1),
                                QE * eoq + s * h0:QE * eoq + (s + 1) * h0],
                        in_=fo[:, s * h0:(s + 1) * h0],
                    )
        for i, ti in enumerate(tis[2:]):
            emit_ffn(
                ti,
                pool=(ps_acc if i % 2 == 0 else ps_s),
                tag=("ps_acc" if i % 2 == 0 else "sp"),
            )

    nc.finalize()
    return nc


_NC_CACHE = {}


def _get_nc(shape_key):
    if shape_key not in _NC_CACHE:
        _NC_CACHE[shape_key] = build_nc(*shape_key)
    return _NC_CACHE[shape_key]


def kernel(x, Wq, Wk, Wv, Wo, bo, W1, b1):
    x = np.ascontiguousarray(np.asarray(x, dtype=np.float32))
    ws = {
        "Wq": np.ascontiguousarray(np.asarray(Wq, dtype=np.float32)),
        "Wk": np.ascontiguousarray(np.asarray(Wk, dtype=np.float32)),
        "Wv": np.ascontiguousarray(np.asarray(Wv, dtype=np.float32)),
        "Wo": np.ascontiguousarray(np.asarray(Wo, dtype=np.float32)),
        "bo": np.ascontiguousarray(np.asarray(bo, dtype=np.float32)),
        "W1": np.ascontiguousarray(np.asarray(W1, dtype=np.float32)),
        "b1": np.ascontiguousarray(np.asarray(b1, dtype=np.float32)),
    }
    B, TT, EE = x.shape
    assert B == N_CORES
    nc = _get_nc((TT, EE, H, DH))
    in_maps = [dict(ws, x=x[b]) for b in range(B)]
    res = run_bass_kernel_spmd(nc, in_maps, core_ids=list(range(N_CORES)))
    return np.stack([res.results[b]["out"] for b in range(B)], axis=0).astype(
        np.float32
    )
